# revision 6
# baseline (speedup 1.0000x reference)
"""Trainium2 Bass kernel for nn_AMLNeuralNetwork3D — row-split L1 variant.

L1 is ROW-split (contraction over the core's own 1024 genes), so it needs
no collective before it: the PE starts real work ~35us in, while the
CC-channel init barrier (~65us) completes in the background.  L1 produces
full-height partials [8192, NB] which are ReduceScatter'd (bf16) to the
core's feature slice, bias+relu applied, then AllGather'd — after which
L2/L3 proceed column-split exactly as the baseline.

Tail: the last chunk of L3 runs in two feature-half passes so the final
PSUM drain overlaps the second pass.
"""

import sys

if "/opt/trn_rl_repo" not in sys.path:
    sys.path.insert(0, "/opt/trn_rl_repo")

import numpy as np
import ml_dtypes

N_CORES = 8
G = 8192
B = 1024
L = 4
GS = G // N_CORES
NB = 512
NCHUNK = B // NB
GT = GS // 128    # 8 gene tiles per core slice / k-tiles for row-split L1
KT = G // 128     # 64 out-feature tiles for row-split L1 / k-tiles for L2,L3

BF16 = ml_dtypes.bfloat16

_compiled = {}

N_WARMUP = 130


def _build_graph():
    from concourse import bacc, tile
    from concourse.tile_rust import add_dep_helper
    import concourse.mybir as mybir

    fp32 = mybir.dt.float32
    bf16 = mybir.dt.bfloat16
    Relu = mybir.ActivationFunctionType.Relu
    Copy = mybir.ActivationFunctionType.Copy
    mult = mybir.AluOpType.mult
    add = mybir.AluOpType.add
    bypass = mybir.AluOpType.bypass

    nc = bacc.Bacc(None, target_bir_lowering=False, num_devices=N_CORES)

    x_p = nc.declare_dram_parameter("x", [L, GS, B], bf16, isOutput=False)
    # per-feature scalars: cols 0..3 = W_local, 4 = b_local, 5..7 = b1..b3
    scal_p = nc.declare_dram_parameter("scal", [GS, 8], fp32, isOutput=False)
    # w1t: row-split tiled [m, p, g*128+c] = W1[m*128+c, own_slice_g*128+p]
    w1_p = nc.declare_dram_parameter("w1t", [KT, 128, GS], bf16, isOutput=False)
    w_p = {
        k: nc.declare_dram_parameter(f"w{k}t", [G, GS], bf16, isOutput=False)
        for k in (2, 3)
    }
    out_p = nc.declare_dram_parameter("out", [GS, B], fp32, isOutput=True)

    rg = [list(range(N_CORES))]

    with tile.TileContext(nc) as tc:
        with (
            tc.tile_pool(name="dram", bufs=1, space="DRAM") as dram,
            tc.tile_pool(name="scal", bufs=GT) as spool,
            tc.tile_pool(name="xin", bufs=10) as xpool,
            tc.tile_pool(name="loc", bufs=6) as lpool,
            tc.tile_pool(name="h0p", bufs=GT) as h0pool,
            tc.tile_pool(name="hin", bufs=32) as hpool,
            tc.tile_pool(name="wblk", bufs=24) as wpool,
            tc.tile_pool(name="hout", bufs=10) as opool,
            tc.tile_pool(name="psum", bufs=8, space="PSUM") as ppool,
        ):
            _gath_space = "Shared"
            # L1 partial buffers + ReduceScatter outputs (per batch chunk)
            part = [
                dram.tile([G, NB], bf16, name=f"part{j}", tag=f"part{j}")
                for j in range(NCHUNK)
            ]
            rsout = [
                dram.tile([GS, NB], bf16, name=f"rsout{j}", tag=f"rsout{j}")
                for j in range(NCHUNK)
            ]
            # transitions 1,2 (after L1/L2): one AG per batch chunk
            slc = [
                [
                    dram.tile([GS, NB], bf16, name=f"slc_{t}_{j}", tag=f"slc_{t}_{j}")
                    for j in range(NCHUNK)
                ]
                for t in range(2)
            ]
            gath = [
                [
                    dram.tile(
                        [G, NB], bf16, name=f"gath_{t}_{j}", tag=f"gath_{t}_{j}",
                        addr_space=_gath_space,
                    )
                    for j in range(NCHUNK)
                ]
                for t in range(2)
            ]

            # --- PE warmup bridges until the local layer's h0 is ready
            wu_w = spool.tile([128, 128], bf16, name="wu_w", tag="wu_w")
            nc.sync.dma_start(wu_w[:], w_p[2][0:128, 0:128])
            wu_h = spool.tile([128, NB], bf16, name="wu_h", tag="wu_h")
            nc.sync.dma_start(wu_h[:], w_p[2][0:128, 0:NB])
            wu_ps = ppool.tile([128, NB], fp32, name="wu_ps", tag="ps")
            for i in range(N_WARMUP):
                nc.tensor.matmul(
                    wu_ps[:], wu_w[:], wu_h[:],
                    start=(i == 0), stop=(i == N_WARMUP - 1),
                )
            wu_out = spool.tile([128, NB], bf16, name="wu_out", tag="wu_out")
            nc.scalar.activation(wu_out[:], wu_ps[:], Copy)
            wu_dram = dram.tile([128, NB], bf16, name="wu_dram", tag="wu_dram")
            nc.scalar.dma_start(wu_dram[:], wu_out[:])

            sc = []
            for gt in range(GT):
                s = spool.tile([128, 8], fp32, name=f"sc{gt}", tag="sc")
                nc.sync.dma_start(s[:], scal_p[gt * 128 : (gt + 1) * 128, :])
                sc.append(s)

            # ---- local layer on full-batch [128, B] tiles ----
            acts = []
            h0 = []
            for gt in range(GT):
                xt = []
                for l in range(L):
                    t = xpool.tile([128, B], bf16, name=f"x{gt}_{l}", tag="x")
                    nc.sync.dma_start(t[:], x_p[l, gt * 128 : (gt + 1) * 128, :])
                    xt.append(t)
                acc = lpool.tile([128, B], bf16, name=f"a{gt}_0", tag="acc")
                nc.vector.tensor_scalar(
                    acc[:], xt[0][:], sc[gt][:, 0:1], None, mult
                )
                for l in range(1, L):
                    acc2 = lpool.tile([128, B], bf16, name=f"a{gt}_{l}", tag="acc")
                    nc.vector.scalar_tensor_tensor(
                        acc2[:], xt[l][:], sc[gt][:, l : l + 1], acc[:], mult, add
                    )
                    acc = acc2
                h = h0pool.tile([128, B], bf16, name=f"h0_{gt}", tag="h0")
                ai = nc.scalar.activation(h[:], acc[:], Relu, bias=sc[gt][:, 4:5])
                acts.append(ai)
                h0.append(h)

            def l1_rowsplit(j):
                # partial[m*128+c, b] = sum_g w1t[m][:, g] . h0[g][:, chunk j]
                wdmas = {}
                NPREF = 8
                for m in range(NPREF):
                    wb = wpool.tile([128, GS], bf16, name=f"w1_{j}_{m}", tag="wblk")
                    wdma = nc.sync.dma_start(wb[:], w1_p[m, :, :])
                    if j == 0:
                        add_dep_helper(
                            getattr(wdma, "ins", wdma),
                            getattr(acts[3], "ins", acts[3]),
                            reason="x tiles first on HBM",
                        )
                    wdmas[m] = wb
                for m in range(KT):
                    if m + NPREF < KT:
                        mq = m + NPREF
                        wb = wpool.tile(
                            [128, GS], bf16, name=f"w1_{j}_{mq}", tag="wblk"
                        )
                        nc.sync.dma_start(wb[:], w1_p[mq, :, :])
                        wdmas[mq] = wb
                    wb = wdmas.pop(m)
                    pst = ppool.tile([128, NB], fp32, name=f"ps1_{j}_{m}", tag="ps")
                    for g in range(GT):
                        nc.tensor.matmul(
                            pst[:],
                            wb[:, g * 128 : (g + 1) * 128],
                            h0[g][:, j * NB : (j + 1) * NB],
                            start=(g == 0),
                            stop=(g == GT - 1),
                        )
                    pc = opool.tile([128, NB], bf16, name=f"pc{j}_{m}", tag="hout")
                    nc.scalar.activation(pc[:], pst[:], Copy)
                    nc.scalar.dma_start(part[j][m * 128 : (m + 1) * 128, :], pc[:])

            def rs_relu_ag(j):
                nc.gpsimd.collective_compute(
                    "ReduceScatter", add, replica_groups=rg,
                    ins=[part[j][:].opt()], outs=[rsout[j][:].opt()],
                )
                for o in range(GT):
                    rin = hpool.tile([128, NB], bf16, name=f"r{j}_{o}", tag="hin")
                    nc.sync.dma_start(rin[:], rsout[j][o * 128 : (o + 1) * 128, :])
                    ot = opool.tile([128, NB], bf16, name=f"ro{j}_{o}", tag="hout")
                    nc.scalar.activation(ot[:], rin[:], Relu, bias=sc[o][:, 5:6])
                    nc.scalar.dma_start(
                        slc[0][j][o * 128 : (o + 1) * 128, :], ot[:]
                    )
                nc.gpsimd.collective_compute(
                    "AllGather", bypass, replica_groups=rg,
                    ins=[slc[0][j][:].opt()], outs=[gath[0][j][:].opt()],
                )

            def h_dma(k, j, g, ht):
                src = gath[k - 2][j]
                return nc.sync.dma_start(ht[:], src[g * 128 : (g + 1) * 128, :])

            def dense_layer(k, j):
                # k in {2,3}; input from gath[k-2][j]
                wt = w_p[k]
                ps = [
                    ppool.tile([128, NB], fp32, name=f"ps{k}_{j}_{o}", tag="ps")
                    for o in range(GT)
                ]
                wdmas = {}
                NPREF = 8
                for g in range(NPREF):
                    wb = wpool.tile([128, GS], bf16, name=f"w{k}_{j}_{g}", tag="wblk")
                    nc.sync.dma_start(wb[:], wt[g * 128 : (g + 1) * 128, :])
                    wdmas[g] = wb
                for g in range(KT):
                    ht = hpool.tile([128, NB], bf16, name=f"h{k}_{j}_{g}", tag="hin")
                    h_dma(k, j, g, ht)
                    if g + NPREF < KT:
                        gq = g + NPREF
                        wb = wpool.tile(
                            [128, GS], bf16, name=f"w{k}_{j}_{gq}", tag="wblk"
                        )
                        nc.sync.dma_start(wb[:], wt[gq * 128 : (gq + 1) * 128, :])
                        wdmas[gq] = wb
                    wb = wdmas.pop(g)
                    for o in range(GT):
                        nc.tensor.matmul(
                            ps[o][:],
                            wb[:, o * 128 : (o + 1) * 128],
                            ht[:],
                            start=(g == 0),
                            stop=(g == KT - 1),
                        )
                for o in range(GT):
                    if k < 3:
                        ot = opool.tile(
                            [128, NB], bf16, name=f"o{k}_{j}_{o}", tag="hout"
                        )
                        nc.scalar.activation(
                            ot[:], ps[o][:], Relu, bias=sc[o][:, 4 + k : 5 + k]
                        )
                        nc.sync.dma_start(
                            slc[k - 1][j][o * 128 : (o + 1) * 128, :], ot[:]
                        )
                    else:
                        ot = opool.tile(
                            [128, NB], fp32, name=f"o{k}_{j}_{o}", tag="outp"
                        )
                        nc.scalar.activation(
                            ot[:], ps[o][:], Relu, bias=sc[o][:, 7:8]
                        )
                        nc.sync.dma_start(
                            out_p[o * 128 : (o + 1) * 128, j * NB : (j + 1) * NB],
                            ot[:],
                        )

            def dense_layer_last(k, j):
                wt = w_p[k]
                for half in range(2):
                    ps = [
                        ppool.tile([128, NB], fp32, name=f"ps{k}_{j}_{half}_{o}",
                                   tag="ps")
                        for o in range(GT // 2)
                    ]
                    col = half * (GS // 2)
                    for g in range(KT):
                        ht = hpool.tile(
                            [128, NB], bf16, name=f"h{k}_{j}_{half}_{g}", tag="hin"
                        )
                        h_dma(k, j, g, ht)
                        wb = wpool.tile(
                            [128, GS // 2], bf16, name=f"w{k}_{j}_{half}_{g}",
                            tag="wblk",
                        )
                        nc.sync.dma_start(
                            wb[:], wt[g * 128 : (g + 1) * 128, col : col + GS // 2]
                        )
                        for o in range(GT // 2):
                            nc.tensor.matmul(
                                ps[o][:],
                                wb[:, o * 128 : (o + 1) * 128],
                                ht[:],
                                start=(g == 0),
                                stop=(g == KT - 1),
                            )
                    for o in range(GT // 2):
                        oo = half * (GT // 2) + o
                        ot = opool.tile(
                            [128, NB], fp32, name=f"o{k}_{j}_{half}_{o}", tag="outp"
                        )
                        nc.scalar.activation(
                            ot[:], ps[o][:], Relu, bias=sc[oo][:, 7:8]
                        )
                        nc.sync.dma_start(
                            out_p[oo * 128 : (oo + 1) * 128,
                                  j * NB : (j + 1) * NB],
                            ot[:],
                        )

            def allgather(t, j):
                nc.gpsimd.collective_compute(
                    "AllGather", bypass, replica_groups=rg,
                    ins=[slc[t - 1][j][:].opt()], outs=[gath[t - 1][j][:].opt()],
                )

            # emission order = desired overlap order
            l1_rowsplit(0)
            l1_rowsplit(1)
            rs_relu_ag(0)
            rs_relu_ag(1)
            for j in range(NCHUNK):
                dense_layer(2, j)
                allgather(2, j)
            dense_layer(3, 0)
            dense_layer_last(3, 1)

    nc.compile()
    return nc


def _get_nc():
    if "nc" not in _compiled:
        _compiled["nc"] = _build_graph()
    return _compiled["nc"]


def kernel(x, W_local, b_local, W1, b1, W2, b2, W3, b3):
    from concourse.bass_utils import run_bass_kernel_spmd

    nc = _get_nc()

    x = np.asarray(x)
    W1 = np.asarray(W1)
    in_maps = []
    for r in range(N_CORES):
        sl = slice(r * GS, (r + 1) * GS)
        x_r = x[:, :, sl].transpose(0, 2, 1).astype(BF16)
        scal_r = np.concatenate(
            [
                np.asarray(W_local)[sl, :],
                np.asarray(b_local)[sl, None],
                np.asarray(b1)[sl, None],
                np.asarray(b2)[sl, None],
                np.asarray(b3)[sl, None],
            ],
            axis=1,
        ).astype(np.float32)
        # row-split L1 weight tiles: [m, p, g*128+c] = W1[m*128+c, own g*128+p]
        w1s = W1[:, sl].reshape(KT, 128, GT, 128).transpose(0, 3, 2, 1)
        in_maps.append(
            {
                "x": x_r,
                "scal": np.ascontiguousarray(scal_r),
                "w1t": np.ascontiguousarray(w1s.reshape(KT, 128, GS)).astype(BF16),
                "w2t": np.asarray(W2)[sl, :].T.astype(BF16),
                "w3t": np.asarray(W3)[sl, :].T.astype(BF16),
            }
        )

    res = run_bass_kernel_spmd(nc, in_maps, core_ids=list(range(N_CORES)))

    out = np.empty((B, G), np.float32)
    for r in range(N_CORES):
        out[:, r * GS : (r + 1) * GS] = res.results[r]["out"].T
    return out


# revision 18
# speedup vs baseline: 1.0584x; 1.0584x over previous
"""Trainium2 Bass kernel for nn_AMLNeuralNetwork3D — row-split L1 variant.

L1 is ROW-split (contraction over the core's own 1024 genes), so it needs
no collective before it: the PE starts real work ~35us in, while the
CC-channel init barrier (~65us) completes in the background.  L1 produces
full-height partials [8192, NB] which are ReduceScatter'd (bf16) to the
core's feature slice, bias+relu applied, then AllGather'd — after which
L2/L3 proceed column-split exactly as the baseline.

Tail: the last chunk of L3 runs in two feature-half passes so the final
PSUM drain overlaps the second pass.
"""

import sys

if "/opt/trn_rl_repo" not in sys.path:
    sys.path.insert(0, "/opt/trn_rl_repo")

import numpy as np
import ml_dtypes

N_CORES = 8
G = 8192
B = 1024
L = 4
GS = G // N_CORES
NB = 512
NCHUNK = B // NB
GT = GS // 128    # 8 gene tiles per core slice / k-tiles for row-split L1
KT = G // 128     # 64 out-feature tiles for row-split L1 / k-tiles for L2,L3

BF16 = ml_dtypes.bfloat16

_compiled = {}

N_WARMUP = 100


def _build_graph():
    from concourse import bacc, tile
    from concourse.tile_rust import add_dep_helper
    import concourse.mybir as mybir

    fp32 = mybir.dt.float32
    bf16 = mybir.dt.bfloat16
    Relu = mybir.ActivationFunctionType.Relu
    Copy = mybir.ActivationFunctionType.Copy
    mult = mybir.AluOpType.mult
    add = mybir.AluOpType.add
    bypass = mybir.AluOpType.bypass

    nc = bacc.Bacc(None, target_bir_lowering=False, num_devices=N_CORES)

    x_p = nc.declare_dram_parameter("x", [L, GS, B], bf16, isOutput=False)
    # per-feature scalars: cols 0..3 = W_local, 4 = b_local, 5..7 = b1..b3
    scal_p = nc.declare_dram_parameter("scal", [GS, 8], fp32, isOutput=False)
    # w1t: row-split tiled [m, p, g*128+c] = W1[m*128+c, own_slice_g*128+p]
    w1_p = nc.declare_dram_parameter("w1t", [KT, 128, GS], bf16, isOutput=False)
    # full b1 (all 8192 features), tiled [p, g] = b1[g*128+p]: L2's input
    # relu+bias is applied per gathered h tile instead of before the gather
    b1f_p = nc.declare_dram_parameter("b1f", [128, KT], fp32, isOutput=False)
    w_p = {
        k: nc.declare_dram_parameter(f"w{k}t", [G, GS], bf16, isOutput=False)
        for k in (2, 3)
    }
    out_p = nc.declare_dram_parameter("out", [GS, B], fp32, isOutput=True)

    rg = [list(range(N_CORES))]

    with tile.TileContext(nc) as tc:
        with (
            tc.tile_pool(name="dram", bufs=1, space="DRAM") as dram,
            tc.tile_pool(name="scal", bufs=GT) as spool,
            tc.tile_pool(name="xin", bufs=12) as xpool,
            tc.tile_pool(name="loc", bufs=6) as lpool,
            tc.tile_pool(name="h0p", bufs=2 * GT) as h0pool,
            tc.tile_pool(name="hin", bufs=32) as hpool,
            tc.tile_pool(name="wblk", bufs=24) as wpool,
            tc.tile_pool(name="hout", bufs=10) as opool,
            tc.tile_pool(name="pcp", bufs=24) as pcpool,
            tc.tile_pool(name="hact", bufs=16) as hapool,
            tc.tile_pool(name="psum", bufs=8, space="PSUM") as ppool,
        ):
            _gath_space = "Shared"
            # L1 partial buffers + ReduceScatter outputs (per batch chunk)
            part = [
                dram.tile([G, NB], bf16, name=f"part{j}", tag=f"part{j}")
                for j in range(NCHUNK)
            ]
            rsout = [
                dram.tile([GS, NB], bf16, name=f"rsout{j}", tag=f"rsout{j}")
                for j in range(NCHUNK)
            ]
            # transitions 1,2 (after L1/L2): one AG per batch chunk
            slc = [
                [
                    dram.tile([GS, NB], bf16, name=f"slc_{t}_{j}", tag=f"slc_{t}_{j}")
                    for j in range(NCHUNK)
                ]
                for t in range(2)
            ]
            gath = [
                [
                    dram.tile(
                        [G, NB], bf16, name=f"gath_{t}_{j}", tag=f"gath_{t}_{j}",
                        addr_space=_gath_space,
                    )
                    for j in range(NCHUNK)
                ]
                for t in range(2)
            ]

            # --- PE warmup bridges until the local layer's h0 is ready
            wu_w = spool.tile([128, 128], bf16, name="wu_w", tag="wu_w")
            nc.sync.dma_start(wu_w[:], w_p[2][0:128, 0:128])
            wu_h = spool.tile([128, NB], bf16, name="wu_h", tag="wu_h")
            nc.sync.dma_start(wu_h[:], w_p[2][0:128, 0:NB])
            wu_ps = ppool.tile([128, NB], fp32, name="wu_ps", tag="ps")
            for i in range(N_WARMUP):
                nc.tensor.matmul(
                    wu_ps[:], wu_w[:], wu_h[:],
                    start=(i == 0), stop=(i == N_WARMUP - 1),
                )
            wu_out = spool.tile([128, NB], bf16, name="wu_out", tag="wu_out")
            nc.scalar.activation(wu_out[:], wu_ps[:], Copy)
            wu_dram = dram.tile([128, NB], bf16, name="wu_dram", tag="wu_dram")
            nc.scalar.dma_start(wu_dram[:], wu_out[:])

            sc = []
            for gt in range(GT):
                s = spool.tile([128, 8], fp32, name=f"sc{gt}", tag="sc")
                nc.sync.dma_start(s[:], scal_p[gt * 128 : (gt + 1) * 128, :])
                sc.append(s)
            b1f = spool.tile([128, KT], fp32, name="b1f", tag="b1f")
            nc.sync.dma_start(b1f[:], b1f_p[:, :])

            # ---- local layer on per-chunk [128, NB] tiles, chunk 0 first so
            # L1 reaches full rate as early as possible ----
            acts = []
            h0 = [[None] * GT for _ in range(NCHUNK)]
            for j in range(NCHUNK):
                for gt in range(GT):
                    xt = []
                    for l in range(L):
                        t = xpool.tile([128, NB], bf16, name=f"x{j}_{gt}_{l}",
                                       tag="x")
                        nc.sync.dma_start(
                            t[:],
                            x_p[l, gt * 128 : (gt + 1) * 128,
                                j * NB : (j + 1) * NB],
                        )
                        xt.append(t)
                    acc = lpool.tile([128, NB], bf16, name=f"a{j}_{gt}_0",
                                     tag="acc")
                    nc.vector.tensor_scalar(
                        acc[:], xt[0][:], sc[gt][:, 0:1], None, mult
                    )
                    for l in range(1, L):
                        acc2 = lpool.tile([128, NB], bf16,
                                          name=f"a{j}_{gt}_{l}", tag="acc")
                        nc.vector.scalar_tensor_tensor(
                            acc2[:], xt[l][:], sc[gt][:, l : l + 1], acc[:],
                            mult, add
                        )
                        acc = acc2
                    h = h0pool.tile([128, NB], bf16, name=f"h0_{j}_{gt}",
                                    tag="h0")
                    ai = nc.scalar.activation(h[:], acc[:], Relu,
                                              bias=sc[gt][:, 4:5])
                    if j == 0:
                        acts.append(ai)
                    h0[j][gt] = h

            def l1_rowsplit(j):
                # partial[m*128+c, b] = sum_g w1t[m][:, g] . h0[g][:, chunk j]
                wdmas = {}
                NPREF = 8
                for m in range(NPREF):
                    wb = wpool.tile([128, GS], bf16, name=f"w1_{j}_{m}", tag="wblk")
                    wdma = nc.sync.dma_start(wb[:], w1_p[m, :, :])
                    if j == 0:
                        add_dep_helper(
                            getattr(wdma, "ins", wdma),
                            getattr(acts[1], "ins", acts[1]),
                            reason="x tiles first on HBM",
                        )
                    wdmas[m] = wb
                for m in range(KT):
                    if m + NPREF < KT:
                        mq = m + NPREF
                        wb = wpool.tile(
                            [128, GS], bf16, name=f"w1_{j}_{mq}", tag="wblk"
                        )
                        nc.sync.dma_start(wb[:], w1_p[mq, :, :])
                        wdmas[mq] = wb
                    wb = wdmas.pop(m)
                    pst = ppool.tile([128, NB], fp32, name=f"ps1_{j}_{m}", tag="ps")
                    for g in range(GT):
                        nc.tensor.matmul(
                            pst[:],
                            wb[:, g * 128 : (g + 1) * 128],
                            h0[j][g][:],
                            start=(g == 0),
                            stop=(g == GT - 1),
                        )
                    # PSUM-freeing copy on the otherwise-idle Vector engine;
                    # the part write goes via Scalar, where a backlog (when a
                    # concurrent ReduceScatter hogs the DMA engines) blocks
                    # only further part writes — opool depth is the elasticity
                    pc = pcpool.tile([128, NB], bf16, name=f"pc{j}_{m}", tag="pcp")
                    nc.vector.tensor_scalar(pc[:], pst[:], 1.0, None, mult)
                    nc.scalar.dma_start(part[j][m * 128 : (m + 1) * 128, :], pc[:])

            cc_chain = []

            def chain(cc):
                # force GpSimd trigger order = emission order so the serial
                # CC stream can't reorder (a late RS ahead of a ready AG)
                if cc_chain:
                    add_dep_helper(
                        getattr(cc, "ins", cc),
                        getattr(cc_chain[-1], "ins", cc_chain[-1]),
                        reason="cc stream order",
                    )
                cc_chain.append(cc)

            def rs_ag(j):
                # ReduceScatter the raw partials, then AllGather the raw
                # reduced slice immediately; bias+relu is applied on the
                # gathered tiles as L2 loads them
                cc = nc.gpsimd.collective_compute(
                    "ReduceScatter", add, replica_groups=rg,
                    ins=[part[j][:].opt()], outs=[rsout[j][:].opt()],
                )
                chain(cc)
                cc = nc.gpsimd.collective_compute(
                    "AllGather", bypass, replica_groups=rg,
                    ins=[rsout[j][:].opt()], outs=[gath[0][j][:].opt()],
                )
                chain(cc)

            def h_dma(k, j, g, ht):
                src = gath[k - 2][j]
                return nc.sync.dma_start(ht[:], src[g * 128 : (g + 1) * 128, :])

            def dense_layer(k, j):
                # k in {2,3}; input from gath[k-2][j]
                wt = w_p[k]
                ps = [
                    ppool.tile([128, NB], fp32, name=f"ps{k}_{j}_{o}", tag="ps")
                    for o in range(GT)
                ]
                wdmas = {}
                NPREF = 8
                for g in range(NPREF):
                    wb = wpool.tile([128, GS], bf16, name=f"w{k}_{j}_{g}", tag="wblk")
                    nc.sync.dma_start(wb[:], wt[g * 128 : (g + 1) * 128, :])
                    wdmas[g] = wb
                for g in range(KT):
                    ht = hpool.tile([128, NB], bf16, name=f"h{k}_{j}_{g}", tag="hin")
                    h_dma(k, j, g, ht)
                    if g + NPREF < KT:
                        gq = g + NPREF
                        wb = wpool.tile(
                            [128, GS], bf16, name=f"w{k}_{j}_{gq}", tag="wblk"
                        )
                        nc.sync.dma_start(wb[:], wt[gq * 128 : (gq + 1) * 128, :])
                        wdmas[gq] = wb
                    if k == 2:
                        # h is the raw (pre-activation) gathered L1 output
                        ha = hapool.tile(
                            [128, NB], bf16, name=f"ha{j}_{g}", tag="hact"
                        )
                        nc.scalar.activation(
                            ha[:], ht[:], Relu, bias=b1f[:, g : g + 1]
                        )
                        ht = ha
                    wb = wdmas.pop(g)
                    for o in range(GT):
                        nc.tensor.matmul(
                            ps[o][:],
                            wb[:, o * 128 : (o + 1) * 128],
                            ht[:],
                            start=(g == 0),
                            stop=(g == KT - 1),
                        )
                for o in range(GT):
                    if k < 3:
                        ot = opool.tile(
                            [128, NB], bf16, name=f"o{k}_{j}_{o}", tag="hout"
                        )
                        nc.scalar.activation(
                            ot[:], ps[o][:], Relu, bias=sc[o][:, 4 + k : 5 + k]
                        )
                        nc.sync.dma_start(
                            slc[k - 1][j][o * 128 : (o + 1) * 128, :], ot[:]
                        )
                    else:
                        ot = opool.tile(
                            [128, NB], fp32, name=f"o{k}_{j}_{o}", tag="outp"
                        )
                        nc.scalar.activation(
                            ot[:], ps[o][:], Relu, bias=sc[o][:, 7:8]
                        )
                        nc.sync.dma_start(
                            out_p[o * 128 : (o + 1) * 128, j * NB : (j + 1) * NB],
                            ot[:],
                        )

            def dense_layer_last(k, j):
                wt = w_p[k]
                for half in range(2):
                    ps = [
                        ppool.tile([128, NB], fp32, name=f"ps{k}_{j}_{half}_{o}",
                                   tag="ps")
                        for o in range(GT // 2)
                    ]
                    col = half * (GS // 2)
                    for g in range(KT):
                        ht = hpool.tile(
                            [128, NB], bf16, name=f"h{k}_{j}_{half}_{g}", tag="hin"
                        )
                        h_dma(k, j, g, ht)
                        wb = wpool.tile(
                            [128, GS // 2], bf16, name=f"w{k}_{j}_{half}_{g}",
                            tag="wblk",
                        )
                        nc.sync.dma_start(
                            wb[:], wt[g * 128 : (g + 1) * 128, col : col + GS // 2]
                        )
                        for o in range(GT // 2):
                            nc.tensor.matmul(
                                ps[o][:],
                                wb[:, o * 128 : (o + 1) * 128],
                                ht[:],
                                start=(g == 0),
                                stop=(g == KT - 1),
                            )
                    for o in range(GT // 2):
                        oo = half * (GT // 2) + o
                        ot = opool.tile(
                            [128, NB], fp32, name=f"o{k}_{j}_{half}_{o}", tag="outp"
                        )
                        nc.scalar.activation(
                            ot[:], ps[o][:], Relu, bias=sc[oo][:, 7:8]
                        )
                        nc.sync.dma_start(
                            out_p[oo * 128 : (oo + 1) * 128,
                                  j * NB : (j + 1) * NB],
                            ot[:],
                        )

            def allgather(t, j):
                cc = nc.gpsimd.collective_compute(
                    "AllGather", bypass, replica_groups=rg,
                    ins=[slc[t - 1][j][:].opt()], outs=[gath[t - 1][j][:].opt()],
                )
                chain(cc)

            # emission order = desired overlap order
            l1_rowsplit(0)
            l1_rowsplit(1)
            rs_ag(0)
            rs_ag(1)
            for j in range(NCHUNK):
                dense_layer(2, j)
                allgather(2, j)
            dense_layer(3, 0)
            dense_layer_last(3, 1)

    nc.compile()
    return nc


def _get_nc():
    if "nc" not in _compiled:
        _compiled["nc"] = _build_graph()
    return _compiled["nc"]


def kernel(x, W_local, b_local, W1, b1, W2, b2, W3, b3):
    from concourse.bass_utils import run_bass_kernel_spmd

    nc = _get_nc()

    x = np.asarray(x)
    W1 = np.asarray(W1)
    in_maps = []
    for r in range(N_CORES):
        sl = slice(r * GS, (r + 1) * GS)
        x_r = x[:, :, sl].transpose(0, 2, 1).astype(BF16)
        scal_r = np.concatenate(
            [
                np.asarray(W_local)[sl, :],
                np.asarray(b_local)[sl, None],
                np.asarray(b1)[sl, None],
                np.asarray(b2)[sl, None],
                np.asarray(b3)[sl, None],
            ],
            axis=1,
        ).astype(np.float32)
        # row-split L1 weight tiles: [m, p, g*128+c] = W1[m*128+c, own g*128+p]
        w1s = W1[:, sl].reshape(KT, 128, GT, 128).transpose(0, 3, 2, 1)
        in_maps.append(
            {
                "x": x_r,
                "scal": np.ascontiguousarray(scal_r),
                "w1t": np.ascontiguousarray(w1s.reshape(KT, 128, GS)).astype(BF16),
                "b1f": np.ascontiguousarray(
                    np.asarray(b1).reshape(KT, 128).T
                ).astype(np.float32),
                "w2t": np.asarray(W2)[sl, :].T.astype(BF16),
                "w3t": np.asarray(W3)[sl, :].T.astype(BF16),
            }
        )

    res = run_bass_kernel_spmd(nc, in_maps, core_ids=list(range(N_CORES)))

    out = np.empty((B, G), np.float32)
    for r in range(N_CORES):
        out[:, r * GS : (r + 1) * GS] = res.results[r]["out"].T
    return out


# revision 19
# speedup vs baseline: 1.1503x; 1.0868x over previous
"""Trainium2 Bass kernel for nn_AMLNeuralNetwork3D (dense_mlp).

Strategy: 8-way tensor parallel (column split on output features) for all
three 8192x8192 dense layers; the per-gene local layer shards along the
gene axis (matching the feature split).  After the local layer and after
L1/L2 the per-core feature slices are AllGather'd (concat on partition
axis = gene axis).  L3 slices are returned per-core and assembled on host.

Layout: activations are kept feature-major [features, batch] on chip so a
layer's output layout equals the next layer's input layout (contraction is
over the partition axis on the TensorEngine).  Weights are pre-transposed
on host to [in_features, out_slice] so all DMAs are wide/contiguous.

Compute in bf16 (full-rate on the PE, fp32 PSUM accumulation); measured
L2 rel-err of the full net in bf16 is ~5e-3.
"""

import sys

if "/opt/trn_rl_repo" not in sys.path:
    sys.path.insert(0, "/opt/trn_rl_repo")

import numpy as np
import ml_dtypes

N_CORES = 8
G = 8192          # genes / features
B = 1024          # batch
L = 4             # levels
GS = G // N_CORES # per-core feature slice (1024)
NB = 512          # batch chunk (one PSUM bank at fp32)
NCHUNK = B // NB  # 2
GT = GS // 128    # gene tiles per core slice (8)
KT = G // 128     # contraction tiles (64)

BF16 = ml_dtypes.bfloat16

_compiled = {}

# gathered-feature order when the first AllGather is split into two
# feature halves: [core0 f0:512, core1 f1024:1536, ...] then the second halves
_PERM_HALVES = np.concatenate(
    [np.arange(r * 1024, r * 1024 + 512) for r in range(8)]
    + [np.arange(r * 1024 + 512, (r + 1) * 1024) for r in range(8)]
)


def _build_graph():
    from concourse import bacc, tile
    from concourse.tile_rust import add_dep_helper
    import concourse.mybir as mybir

    fp32 = mybir.dt.float32
    bf16 = mybir.dt.bfloat16
    Relu = mybir.ActivationFunctionType.Relu
    mult = mybir.AluOpType.mult
    add = mybir.AluOpType.add
    bypass = mybir.AluOpType.bypass

    nc = bacc.Bacc(None, target_bir_lowering=False, num_devices=N_CORES)

    # ---- parameters (per-core shards; same graph on all cores) ----
    x_p = nc.declare_dram_parameter("x", [L, GS, B], bf16, isOutput=False)
    # per-feature scalars: cols 0..3 = W_local, 4 = b_local, 5..7 = b1..b3
    scal_p = nc.declare_dram_parameter("scal", [GS, 8], fp32, isOutput=False)
    w_p = [
        nc.declare_dram_parameter(f"w{k}t", [G, GS], bf16, isOutput=False)
        for k in (1, 2, 3)
    ]
    out_p = nc.declare_dram_parameter("out", [GS, B], fp32, isOutput=True)

    rg = [list(range(N_CORES))]

    with tile.TileContext(nc) as tc:
        with (
            tc.tile_pool(name="dram", bufs=1, space="DRAM") as dram,
            tc.tile_pool(name="scal", bufs=GT) as spool,
            tc.tile_pool(name="xin", bufs=12) as xpool,
            tc.tile_pool(name="loc", bufs=10) as lpool,
            tc.tile_pool(name="hin", bufs=28) as hpool,
            tc.tile_pool(name="wblk", bufs=16) as wpool,
            tc.tile_pool(name="hout", bufs=6) as opool,
            tc.tile_pool(name="psum", bufs=8, space="PSUM") as ppool,
        ):
            # bounce buffers for the 3 AllGather transitions x 2 chunks
            slc = [
                [
                    dram.tile([GS, NB], bf16, name=f"slc_{t}_{j}", tag=f"slc_{t}_{j}")
                    for j in range(NCHUNK)
                ]
                for t in range(3)
            ]
            _gath_space = "Shared"
            gath = [
                [
                    dram.tile(
                        [G, NB], bf16, name=f"gath_{t}_{j}", tag=f"gath_{t}_{j}",
                        addr_space=_gath_space,
                    )
                    for j in range(NCHUNK)
                ]
                for t in range(3)
            ]
            # transition-0 chunk-0 AllGather is split along the feature axis:
            # layer 1 starts accumulating K as soon as the first half lands.
            # (w1t rows are permuted on host to match the half-major order.)
            slc0h = [
                [
                    dram.tile(
                        [GS // 2, NB], bf16, name=f"slc0h{j}_{a}",
                        tag=f"slc0h{j}_{a}",
                    )
                    for a in range(2)
                ]
                for j in range(NCHUNK)
            ]
            gath0h = [
                [
                    dram.tile(
                        [G // 2, NB], bf16, name=f"gath0h{j}_{a}",
                        tag=f"gath0h{j}_{a}", addr_space=_gath_space,
                    )
                    for a in range(2)
                ]
                for j in range(NCHUNK)
            ]

            # --- PE warmup: the PE would otherwise idle until the first
            # gathered tiles arrive (~90us: launch barrier + the first
            # AllGather); dummy matmuls keep the HAM clock-gate warm
            # through the prologue at zero cost.
            wu_w = spool.tile([128, 128], bf16, name="wu_w", tag="wu_w")
            nc.sync.dma_start(wu_w[:], w_p[0][0:128, 0:128])
            wu_h = spool.tile([128, NB], bf16, name="wu_h", tag="wu_h")
            nc.sync.dma_start(wu_h[:], w_p[0][0:128, 0:NB])
            wu_ps = ppool.tile([128, NB], fp32, name="wu_ps", tag="ps")
            N_WARMUP = 290
            wu_gate = None
            for i in range(N_WARMUP):
                mi = nc.tensor.matmul(
                    wu_ps[:], wu_w[:], wu_h[:],
                    start=(i == 0), stop=(i == N_WARMUP - 1),
                )
                if i == 64:
                    wu_gate = mi
            wu_out = spool.tile([128, NB], bf16, name="wu_out", tag="wu_out")
            nc.scalar.activation(
                wu_out[:], wu_ps[:], mybir.ActivationFunctionType.Copy
            )
            wu_dram = dram.tile([128, NB], bf16, name="wu_dram", tag="wu_dram")
            nc.sync.dma_start(wu_dram[:], wu_out[:])

            # per-feature scalar tiles, persistent
            sc = []
            for gt in range(GT):
                s = spool.tile([128, 8], fp32, name=f"sc{gt}", tag="sc")
                nc.sync.dma_start(s[:], scal_p[gt * 128 : (gt + 1) * 128, :])
                sc.append(s)

            def local_layer(j):
                # returns the slc-write DMA instructions for optional gating
                slc_writes = []
                for gt in range(GT):
                    xt = []
                    for l in range(L):
                        t = xpool.tile([128, NB], bf16, name=f"x{j}_{gt}_{l}", tag="x")
                        nc.sync.dma_start(
                            t[:],
                            x_p[l, gt * 128 : (gt + 1) * 128, j * NB : (j + 1) * NB],
                        )
                        xt.append(t)
                    acc = lpool.tile([128, NB], bf16, name=f"a{j}_{gt}_0", tag="acc")
                    nc.vector.tensor_scalar(
                        acc[:], xt[0][:], sc[gt][:, 0:1], None, mult
                    )
                    for l in range(1, L):
                        acc2 = lpool.tile(
                            [128, NB], bf16, name=f"a{j}_{gt}_{l}", tag="acc"
                        )
                        nc.vector.scalar_tensor_tensor(
                            acc2[:], xt[l][:], sc[gt][:, l : l + 1], acc[:], mult, add
                        )
                        acc = acc2
                    h0 = opool.tile([128, NB], bf16, name=f"h0_{j}_{gt}", tag="hout")
                    nc.scalar.activation(h0[:], acc[:], Relu, bias=sc[gt][:, 4:5])
                    a, row = gt // 4, (gt % 4) * 128
                    # chunk-1 writes are gated on an L1 matmul marker; keep
                    # them off the sync ring so they can't head-of-line block
                    # L1's h-tile stream behind that gate
                    eng = nc.sync if j == 0 else nc.scalar
                    w = eng.dma_start(
                        slc0h[j][a][row : row + 128, :], h0[:]
                    )
                    slc_writes.append(w)
                    if j == 0 and gt == 3:
                        nc.gpsimd.collective_compute(
                            "AllGather", bypass, replica_groups=rg,
                            ins=[slc0h[0][0][:].opt()],
                            outs=[gath0h[0][0][:].opt()],
                        )
                return slc_writes

            def dense_layer(k, j):
                # k in {1,2,3}; input from gath[k-1][j]; output slice ->
                # slc[k][j] (k<3) or out_p (k==3)
                marker = {}
                src = gath[k - 1][j]
                wt = w_p[k - 1]
                halves = k == 1
                ps = [
                    ppool.tile([128, NB], fp32, name=f"ps{k}_{j}_{o}", tag="ps")
                    for o in range(GT)
                ]
                for g in range(KT):
                    ht = hpool.tile([128, NB], bf16, name=f"h{k}_{j}_{g}", tag="hin")
                    if halves:
                        hsrc = gath0h[j][g // (KT // 2)]
                        row = (g % (KT // 2)) * 128
                        nc.sync.dma_start(ht[:], hsrc[row : row + 128, :])
                    else:
                        nc.sync.dma_start(ht[:], src[g * 128 : (g + 1) * 128, :])
                    wb = wpool.tile([128, GS], bf16, name=f"w{k}_{j}_{g}", tag="wblk")
                    wdma = nc.sync.dma_start(wb[:], wt[g * 128 : (g + 1) * 128, :])
                    if k == 1 and j == 0 and g < 20:
                        # keep the W prefetch burst behind the local layer's
                        # x tiles on the sync ring
                        add_dep_helper(
                            getattr(wdma, "ins", wdma),
                            getattr(wu_gate, "ins", wu_gate),
                            reason="delay W prefetch past x tiles",
                        )
                    for o in range(GT):
                        mmi = nc.tensor.matmul(
                            ps[o][:],
                            wb[:, o * 128 : (o + 1) * 128],
                            ht[:],
                            start=(g == 0),
                            stop=(g == KT - 1),
                        )
                    marker[g] = mmi
                for o in range(GT):
                    if k < 3:
                        ot = opool.tile(
                            [128, NB], bf16, name=f"o{k}_{j}_{o}", tag="hout"
                        )
                        nc.scalar.activation(
                            ot[:], ps[o][:], Relu, bias=sc[o][:, 4 + k : 5 + k]
                        )
                        nc.sync.dma_start(
                            slc[k][j][o * 128 : (o + 1) * 128, :], ot[:]
                        )
                    else:
                        ot = opool.tile(
                            [128, NB], fp32, name=f"o{k}_{j}_{o}", tag="outp"
                        )
                        nc.scalar.activation(
                            ot[:], ps[o][:], Relu, bias=sc[o][:, 7:8]
                        )
                        nc.sync.dma_start(
                            out_p[o * 128 : (o + 1) * 128, j * NB : (j + 1) * NB],
                            ot[:],
                        )
                return marker

            def allgather(t, j):
                nc.gpsimd.collective_compute(
                    "AllGather",
                    bypass,
                    replica_groups=rg,
                    ins=[slc[t][j][:].opt()],
                    outs=[gath[t][j][:].opt()],
                )

            # emission order = desired overlap order
            local_layer(0)
            nc.gpsimd.collective_compute(
                "AllGather", bypass, replica_groups=rg,
                ins=[slc0h[0][1][:].opt()], outs=[gath0h[0][1][:].opt()],
            )
            slc1_writes = local_layer(1)
            for a in range(2):
                nc.gpsimd.collective_compute(
                    "AllGather", bypass, replica_groups=rg,
                    ins=[slc0h[1][a][:].opt()], outs=[gath0h[1][a][:].opt()],
                )
            first = True
            for k in (1, 2, 3):
                for j in range(NCHUNK):
                    marker = dense_layer(k, j)
                    if first:
                        # AG(0,c1)'s transfer otherwise collides with L1-c0's
                        # h/W DMA ramp-up right after the quarter-gathers;
                        # hold it until L1-c0 is ~1/3 done
                        gate = marker[4]
                        for w in slc1_writes:
                            add_dep_helper(
                                getattr(w, "ins", w),
                                getattr(gate, "ins", gate),
                                reason="defer AG(0,c1) past L1 ramp",
                            )
                        first = False
                    if k < 3:
                        allgather(k, j)

    nc.compile()
    return nc


def _get_nc():
    if "nc" not in _compiled:
        _compiled["nc"] = _build_graph()
    return _compiled["nc"]


def kernel(x, W_local, b_local, W1, b1, W2, b2, W3, b3):
    from concourse.bass_utils import run_bass_kernel_spmd

    nc = _get_nc()

    x = np.asarray(x)
    in_maps = []
    for r in range(N_CORES):
        sl = slice(r * GS, (r + 1) * GS)
        x_r = x[:, :, sl].transpose(0, 2, 1).astype(BF16)
        scal_r = np.concatenate(
            [
                np.asarray(W_local)[sl, :],
                np.asarray(b_local)[sl, None],
                np.asarray(b1)[sl, None],
                np.asarray(b2)[sl, None],
                np.asarray(b3)[sl, None],
            ],
            axis=1,
        ).astype(np.float32)
        in_maps.append(
            {
                "x": x_r,
                "scal": np.ascontiguousarray(scal_r),
                "w1t": np.asarray(W1)[sl, :].T.astype(BF16)[_PERM_HALVES, :],
                "w2t": np.asarray(W2)[sl, :].T.astype(BF16),
                "w3t": np.asarray(W3)[sl, :].T.astype(BF16),
            }
        )

    res = run_bass_kernel_spmd(nc, in_maps, core_ids=list(range(N_CORES)))

    out = np.empty((B, G), np.float32)
    for r in range(N_CORES):
        out[:, r * GS : (r + 1) * GS] = res.results[r]["out"].T
    return out

